# revision 3
# baseline (speedup 1.0000x reference)
"""Trainium2 Bass kernel for nn_DeepWF_18820546691332 (FermiNet-style deep
wavefunction, single electron configuration: 512 electrons, 32 atoms).

Structure of the computation (matching reference semantics):
  1. encoder + 4 FermiLayers -> h_one (512, 256)          [host, ~1.8 GFLOP]
  2. all-pairs antisymmetrizer over each 256-electron spin block
     (32640 ordered pairs x 2 orders x 256x256 MLP = ~34 GFLOP dense)
     via the U/V factorization of the pair-MLP first layer:
        mlp1([h_i, h_j]) layer1 = silu(U[i] + V[j]),
        U = h @ W1_top + b1, V = h @ W1_bot
     The per-channel product prod_p hid[p, c] over the 32640 pairs is
     evaluated in log-domain: S_c = sum ln|hid|, n_neg_c = #negatives.
     This stage runs on the 8 NeuronCores (this file's Bass kernel),
     pair-rows strided across cores.                       [device]
  3. a_c = (-1)^n_neg * exp(S_c) (reproducing the f32 overflow-to-inf /
     underflow-to-0 fate of the reference's jnp.prod exactly: every
     channel's |S_c| margin is in the hundreds-to-thousands of log units),
     then A = mlp2(a), out = sum_spins log|A|.             [host, tiny]

The kernel is SPMD: one program, per-core inputs differ only in data
(pair-row permutation + triangle masks).
"""
import numpy as np

import concourse.bass as bass
import concourse.tile as tile
from concourse import mybir, bacc
from concourse.bass_utils import run_bass_kernel_spmd

F32 = mybir.dt.float32
AF = mybir.ActivationFunctionType
ALU = mybir.AluOpType

SILU_GAIN = np.float32(1.7868129431578026)
TANH_GAIN = np.float32(1.5927812698663606)
RSQRT2 = np.float32(2.0 ** -0.5)
NUP = 256                    # electrons per spin block
NCORES = 8
ROWS = NUP // NCORES         # 32 pair-rows (i) per spin per core
BATCH = 4                    # i-rows per dense batch
NB = ROWS // BATCH           # 8 batches per spin
FREE = BATCH * NUP           # 1024 dense free elements per tile row
NPAIR = NUP * (NUP - 1) // 2  # 32640 unordered pairs per spin


# ----------------------------------------------------------------- host math

def _np(x):
    return np.asarray(x, dtype=np.float32)


def _silu(x):
    with np.errstate(over='ignore'):
        return (x / (np.float32(1.0) + np.exp(-x))).astype(np.float32)


def _act_silu(x):
    return _silu(x) * SILU_GAIN


def _act_tanh(x):
    return np.tanh(x).astype(np.float32) * TANH_GAIN


def _mlp(x, layers, act):
    for layer in layers:
        x = act(x @ _np(layer[0]) + _np(layer[1]))
    return x


def _h_one(params, electrons, atoms):
    """Numpy f32 port of the reference encoder + FermiLayer stack."""
    n = electrons.shape[0]
    n_up = NUP

    r_im3 = electrons[:, None, :] - atoms[None, :, :]
    r_im_norm = np.linalg.norm(r_im3, axis=-1, keepdims=True).astype(np.float32)
    r_im4 = np.concatenate([r_im3, r_im_norm], axis=-1)
    W, b = params["enc_dense"]
    h = r_im4 @ _np(W) + _np(b)
    h = (h + _np(params["nuc_emb"])[None]) * RSQRT2
    h_one = _mlp(h, params["enc_mlp"], _act_tanh).mean(1).astype(np.float32)

    eye = np.eye(n, dtype=np.float32)[..., None]
    r_ij = electrons[:, None] - electrons[None]
    r_ij_norm = (np.linalg.norm(r_ij + eye, axis=-1, keepdims=True)
                 * (1.0 - eye)).astype(np.float32)
    h_two = np.concatenate([r_ij, r_ij_norm], axis=-1)

    for layer in params["fermi"]:
        up_mean = h_two[:n_up].mean(0, dtype=np.float32).astype(np.float32)
        dn_mean = h_two[n_up:].mean(0, dtype=np.float32).astype(np.float32)
        one_in = np.concatenate([h_one, up_mean, dn_mean], axis=1)
        g = np.concatenate(
            [h_one[:n_up].mean(0, keepdims=True, dtype=np.float32),
             h_one[n_up:].mean(0, keepdims=True, dtype=np.float32)],
            axis=-1).astype(np.float32)
        Wo, bo = layer["one"]
        Wg = layer["glob"][0]
        h_one_new = _act_silu((one_in @ _np(Wo) + _np(bo) + g @ _np(Wg)) * RSQRT2)
        h_one = ((h_one + h_one_new) * RSQRT2
                 if h_one.shape == h_one_new.shape else h_one_new)
        if "pair" in layer:
            Wp, bp = layer["pair"]
            h_two_new = _act_silu(h_two @ _np(Wp) + _np(bp))
            h_two = ((h_two + h_two_new) * RSQRT2
                     if h_two.shape == h_two_new.shape else h_two_new)
    return h_one.astype(np.float32)


def _core_rows(core):
    return list(range(core, NUP, NCORES))


# ------------------------------------------------------------- device kernel

_NC_CACHE = [None]


def _build_nc():
    if _NC_CACHE[0] is not None:
        return _NC_CACHE[0]
    nc = bacc.Bacc("TRN2", target_bir_lowering=False, debug=False,
                   num_devices=NCORES)
    i_vt = nc.dram_tensor("vt", [2, 2, 128, NUP], F32, kind="ExternalInput")
    i_ut = nc.dram_tensor("ut", [2, 2, 128, NUP], F32, kind="ExternalInput")
    i_utp = nc.dram_tensor("utp", [2, 2, 128, ROWS], F32, kind="ExternalInput")
    i_vtp = nc.dram_tensor("vtp", [2, 2, 128, ROWS], F32, kind="ExternalInput")
    i_w2 = nc.dram_tensor("w2", [2, 2, 128, 128], F32, kind="ExternalInput")
    i_mk = nc.dram_tensor("mask", [NB, FREE], F32, kind="ExternalInput")
    o_s = nc.dram_tensor("o_s", [2, 2, 128, NB], F32, kind="ExternalOutput")
    o_n = nc.dram_tensor("o_n", [2, 2, 128, NB], F32, kind="ExternalOutput")

    def bcast_rows(ap):
        # (1, N) DRAM slice -> broadcast to 128 partitions
        return bass.AP(tensor=ap.tensor, offset=ap.offset,
                       ap=[[0, 128]] + ap.ap[1:])

    def tbcast(tile_ap):
        # (128, J) -> (128, BATCH, J) with 0-stride over BATCH
        return bass.AP(tensor=tile_ap.tensor, offset=tile_ap.offset,
                       ap=[tile_ap.ap[0], [0, BATCH], tile_ap.ap[1]])

    def jbcast(tile_ap):
        # (128, BATCH) -> (128, BATCH, J) with 0-stride over J
        return bass.AP(tensor=tile_ap.tensor, offset=tile_ap.offset,
                       ap=[tile_ap.ap[0], tile_ap.ap[1], [0, NUP]])

    with tile.TileContext(nc) as tc:
        with tc.tile_pool(name="const", bufs=1) as cp, \
             tc.tile_pool(name="acc", bufs=1) as ap_, \
             tc.tile_pool(name="work", bufs=2) as wp, \
             tc.tile_pool(name="psum", bufs=2, space="PSUM") as pp:
            vt_t, ut_t, utp_t, vtp_t = {}, {}, {}, {}
            w2_t = {}
            for s in range(2):
                for c in range(2):
                    vt_t[s, c] = cp.tile([128, NUP], F32, tag=f"vt{s}{c}", name=f"vt{s}{c}")
                    nc.sync.dma_start(out=vt_t[s, c], in_=i_vt.ap()[s, c])
                    ut_t[s, c] = cp.tile([128, NUP], F32, tag=f"ut{s}{c}", name=f"ut{s}{c}")
                    nc.sync.dma_start(out=ut_t[s, c], in_=i_ut.ap()[s, c])
                    utp_t[s, c] = cp.tile([128, ROWS], F32, tag=f"up{s}{c}", name=f"up{s}{c}")
                    nc.sync.dma_start(out=utp_t[s, c], in_=i_utp.ap()[s, c])
                    vtp_t[s, c] = cp.tile([128, ROWS], F32, tag=f"vp{s}{c}", name=f"vp{s}{c}")
                    nc.sync.dma_start(out=vtp_t[s, c], in_=i_vtp.ap()[s, c])
            for c in range(2):
                for k in range(2):
                    w2_t[c, k] = cp.tile([128, 128], F32, tag=f"w{c}{k}", name=f"w{c}{k}")
                    nc.sync.dma_start(out=w2_t[c, k], in_=i_w2.ap()[c, k])

            accS, accN = {}, {}
            for s in range(2):
                for k in range(2):
                    accS[s, k] = ap_.tile([128, NB], F32, tag=f"aS{s}{k}", name=f"aS{s}{k}")
                    accN[s, k] = ap_.tile([128, NB], F32, tag=f"aN{s}{k}", name=f"aN{s}{k}")

            for b in range(NB):
                mk = wp.tile([128, FREE], F32, tag="mk", name="mk")
                nc.sync.dma_start(out=mk, in_=bcast_rows(i_mk.ap()[b:b + 1, :]))
                for s in range(2):
                    # layer-1: z = silu(U_i + V_j) for both orders, both
                    # channel chunks; silu applied in place.
                    z_a, z_b = {}, {}
                    for c in range(2):
                        pa = wp.tile([128, BATCH, NUP], F32, tag=f"pa{c}", name=f"pa{c}")
                        nc.vector.tensor_tensor(
                            out=pa[:], in0=tbcast(vt_t[s, c][:]),
                            in1=jbcast(utp_t[s, c][:, BATCH * b:BATCH * (b + 1)]),
                            op=ALU.add)
                        nc.scalar.activation(out=pa[:], in_=pa[:], func=AF.Silu)
                        z_a[c] = pa
                        pb = wp.tile([128, BATCH, NUP], F32, tag=f"pb{c}", name=f"pb{c}")
                        nc.vector.tensor_tensor(
                            out=pb[:], in0=tbcast(ut_t[s, c][:]),
                            in1=jbcast(vtp_t[s, c][:, BATCH * b:BATCH * (b + 1)]),
                            op=ALU.add)
                        nc.scalar.activation(out=pb[:], in_=pb[:], func=AF.Silu)
                        z_b[c] = pb
                    # layer-2 matmul + silu; log/sign accumulation
                    for k in range(2):
                        sa = wp.tile([128, FREE], F32, tag=f"sa{k}", name=f"sa{k}")
                        sb_ = wp.tile([128, FREE], F32, tag=f"sb{k}", name=f"sb{k}")
                        for zt, st in ((z_a, sa), (z_b, sb_)):
                            pm = pp.tile([128, FREE], F32, tag=f"pm{k}", name=f"pm{k}")
                            for c in range(2):
                                zf = zt[c][:].rearrange("p b j -> p (b j)")
                                for nsub in range(FREE // 512):
                                    sl = slice(512 * nsub, 512 * (nsub + 1))
                                    nc.tensor.matmul(
                                        pm[:, sl], w2_t[c, k][:], zf[:, sl],
                                        start=(c == 0), stop=(c == 1))
                            nc.scalar.activation(out=st[:], in_=pm[:],
                                                 func=AF.Silu)
                        hid = wp.tile([128, FREE], F32, tag=f"h{k}", name=f"h{k}")
                        nc.vector.tensor_tensor(out=hid[:], in0=sa[:],
                                                in1=sb_[:], op=ALU.subtract)
                        # ab = |hid| ; t1 = (ab - 1) * mask ; ln(t1 + 1) accum
                        ab = wp.tile([128, FREE], F32, tag=f"ab{k}", name=f"ab{k}")
                        nc.scalar.activation(out=ab[:], in_=hid[:], func=AF.Abs)
                        t1 = wp.tile([128, FREE], F32, tag=f"t{k}", name=f"t{k}")
                        nc.vector.scalar_tensor_tensor(
                            out=t1[:], in0=ab[:], scalar=1.0, in1=mk[:],
                            op0=ALU.subtract, op1=ALU.mult)
                        jk = wp.tile([128, FREE], F32, tag=f"j{k}", name=f"j{k}")
                        nc.scalar.activation(
                            out=jk[:], in_=t1[:], func=AF.Ln, bias=1.0,
                            accum_out=accS[s, k][:, b:b + 1])
                        # n_neg = sum((hid < 0) * mask)
                        jk2 = wp.tile([128, FREE], F32, tag=f"q{k}", name=f"q{k}")
                        nc.vector.scalar_tensor_tensor(
                            out=jk2[:], in0=hid[:], scalar=0.0, in1=mk[:],
                            op0=ALU.is_lt, op1=ALU.mult,
                            accum_out=accN[s, k][:, b:b + 1])

            for s in range(2):
                for k in range(2):
                    nc.sync.dma_start(out=o_s.ap()[s, k], in_=accS[s, k][:])
                    nc.sync.dma_start(out=o_n.ap()[s, k], in_=accN[s, k][:])

    nc.finalize()
    _NC_CACHE[0] = nc
    return nc


# ------------------------------------------------------------------ driver

def _device_inputs(params, h_one):
    (W1, b1), (W2, _b2) = params["net_mlp1"]
    W1, b1, W2 = _np(W1), _np(b1), _np(W2)
    W2g = (SILU_GAIN * W2).astype(np.float32)

    vt = np.empty((2, 2, 128, NUP), np.float32)
    ut = np.empty((2, 2, 128, NUP), np.float32)
    uv = []
    for s in range(2):
        h = h_one[s * NUP:(s + 1) * NUP]
        U = (h @ W1[:NUP] + b1).astype(np.float32)
        V = (h @ W1[NUP:]).astype(np.float32)
        uv.append((U, V))
        for c in range(2):
            ut[s, c] = U.T[128 * c:128 * (c + 1)]
            vt[s, c] = V.T[128 * c:128 * (c + 1)]
    w2 = np.empty((2, 2, 128, 128), np.float32)
    for c in range(2):
        for k in range(2):
            w2[c, k] = W2g[128 * c:128 * (c + 1), 128 * k:128 * (k + 1)]

    in_maps = []
    jj = np.arange(NUP)
    for core in range(NCORES):
        rows = _core_rows(core)
        utp = np.empty((2, 2, 128, ROWS), np.float32)
        vtp = np.empty((2, 2, 128, ROWS), np.float32)
        for s in range(2):
            for c in range(2):
                utp[s, c] = ut[s, c][:, rows]
                vtp[s, c] = vt[s, c][:, rows]
        mask = np.zeros((NB, BATCH, NUP), np.float32)
        for b in range(NB):
            for t in range(BATCH):
                mask[b, t] = jj > rows[b * BATCH + t]
        in_maps.append({
            "vt": vt, "ut": ut, "utp": utp, "vtp": vtp, "w2": w2,
            "mask": mask.reshape(NB, FREE).copy(),
        })
    return in_maps, uv, W2g


def _finale(params, S_total, nneg_total):
    (W1, b1), (W2, b2) = params["net_mlp2"]
    W1, b1, W2, b2 = _np(W1), _np(b1), _np(W2), _np(b2)
    out = np.float32(0.0)
    for s in range(2):
        S = (S_total[s] + np.float32(NPAIR) * np.log(SILU_GAIN)).astype(np.float32)
        sign = np.where(np.nan_to_num(nneg_total[s]).astype(np.int64) % 2 == 1,
                        np.float32(-1.0), np.float32(1.0))
        with np.errstate(over='ignore', invalid='ignore'):
            a = (sign * np.exp(S)).astype(np.float32)
            z = _act_tanh(a @ W1 + b1)
            A = _act_tanh(z @ W2 + b2)
            out = out + np.log(np.abs(A)).squeeze().astype(np.float32)
    return out


def _to_numpy_tree(obj):
    if isinstance(obj, dict):
        return {k: _to_numpy_tree(v) for k, v in obj.items()}
    if isinstance(obj, (list, tuple)):
        return [_to_numpy_tree(v) for v in obj]
    return _np(obj)


def run_device_partials(params, h_one, collect=None):
    """Compile + run the 8-core bass kernel; reduce partials."""
    in_maps, uv, W2g = _device_inputs(params, h_one)
    nc = _build_nc()
    res = run_bass_kernel_spmd(nc, in_maps, core_ids=list(range(NCORES)))
    S = np.zeros((2, 256), np.float32)
    NN = np.zeros((2, 256), np.float32)
    for core in range(NCORES):
        r = res.results[core]
        for s in range(2):
            for k in range(2):
                S[s, 128 * k:128 * (k + 1)] += r["o_s"][s, k].sum(-1)
                NN[s, 128 * k:128 * (k + 1)] += r["o_n"][s, k].sum(-1)
    if collect is not None:
        collect["per_core"] = [
            {k: np.array(v) for k, v in res.results[c].items()}
            for c in range(NCORES)]
        collect["uv"] = uv
        collect["W2g"] = W2g
    return S, NN


def kernel(params, electrons, atoms):
    params = _to_numpy_tree(params)
    electrons = _np(electrons)
    atoms = _np(atoms)
    h_one = _h_one(params, electrons, atoms)
    S, NN = run_device_partials(params, h_one)
    return np.asarray(_finale(params, S, NN), dtype=np.float32)
